# revision 40
# baseline (speedup 1.0000x reference)
"""Distributed Bass kernel for nn_Attention_32701880992127 on 8 TRN2 NeuronCores.

Sharding (tensor parallel over heads): core d owns q-heads {2d, 2d+1} and
kv-head d//2 (GQA consecutive-repeat mapping). wq/wk/wv are column-split.

The output projection is ROW-split over q: instead of AllGathering the full
attention output to every core (8 MB of wire per core), each core sends core
j its q-slice via AllToAll (0.5 MB of wire), holds the FULL woT in SBUF, and
computes complete output rows y[q_slice, :]. Two AllToAll phases (q 0:1024 /
1024:2048) so phase-a's wo overlaps phase-b's attention; the host interleaves
the per-core row blocks back together.

All matmuls run in bf16 (f32 PSUM accumulation); elementwise math stays f32.
Softmax needs no max-subtraction (qk-norm bounds the scores); the sink
correction folds into the denominator:
    out_h = (sum_k exp(s_qk) v_k) / (exp(sink_h) + sum_k exp(s_qk)).
Scores are computed transposed (ST[k, q]) so exp's output directly feeds the
PV matmul as the moving operand. The causal diagonal 512-block is processed
as 4 k-chunks with shrinking q-windows; only the 128x128 diagonal block of
each chunk needs a (shared triangular) mask, applied on the GpSimd engine.

Engine discipline: the ACT engine runs ONLY Exp and Sqrt (its activation
table cache holds two functions; adding Copy caused a ~1.3us table reload per
proj tile). All PSUM evacuations run on DVE; the three per-st-tile PE
transposes (q0|q1|k) land in one PSUM tile and evacuate with a single DVE
copy. xt streams through a 4-deep buffer ring (frees 8 MB of SBUF for woT).
DMA issue is spread across the sync/scalar/vector/gpsimd queues.
"""
import numpy as np
import ml_dtypes

import concourse.mybir as mybir
import concourse.tile as tile
from concourse import bacc
from concourse.bass_utils import run_bass_kernel_spmd
from concourse.masks import make_identity

dt = mybir.dt
AO = mybir.AluOpType
AF = mybir.ActivationFunctionType
BF16 = ml_dtypes.bfloat16

N_CORES = 8
S = 2048            # sequence length
D = 2048            # model dim
DH = 128            # head dim
HL = 2              # local q heads per core
NC = 16             # d-chunks of 128
NST = 16            # s-tiles of 128
QT = 512            # attention q tile
NQT = S // QT
KC = 128            # attention k chunk
EW = 384            # rope width: q0|q1|k
RMS_EPS = 1.1920929e-07


def build():
    nc = bacc.Bacc("TRN2", target_bir_lowering=False, debug=False, num_devices=N_CORES)

    # all inputs pre-tiled partition-major on the host: [p, ...] with long
    # contiguous per-partition runs
    xt = nc.dram_tensor("xt", [128, NST * NC * 128], dt.bfloat16,
                        kind="ExternalInput").ap()            # [p, st, c, s]
    wqkv = nc.dram_tensor("wqkv", [128, NC * 512], dt.bfloat16,
                          kind="ExternalInput").ap()          # [p, c, e]
    wot = nc.dram_tensor("wot", [128, NC * D], dt.bfloat16,
                         kind="ExternalInput").ap()           # [p, c, dout] FULL woT
    cbar = nc.dram_tensor("cbar", [128, NST * EW], dt.bfloat16,
                          kind="ExternalInput").ap()          # [p, st, e] cos for q0|q1|k
    sbar = nc.dram_tensor("sbar", [128, NST * EW], dt.bfloat16,
                          kind="ExternalInput").ap()          # sign-folded sin
    trimask = nc.dram_tensor("trimask", [KC, KC], dt.bfloat16, kind="ExternalInput").ap()
    esd = nc.dram_tensor("es", [128, HL], dt.float32, kind="ExternalInput").ap()
    # output: 2 phases x 128 q rows, full model dim
    y_out = nc.dram_tensor("y", [2 * 128, D], dt.float32, kind="ExternalOutput").ap()

    with tile.TileContext(nc) as tc:
        with (
            tc.tile_pool(name="const", bufs=1) as cp,
            tc.tile_pool(name="xts", bufs=4) as xp,
            tc.tile_pool(name="work", bufs=2) as wp,
            tc.tile_pool(name="psum", bufs=2, space="PSUM") as pp,
            tc.tile_pool(name="dram", bufs=1, space="DRAM") as dp,
        ):
            # ---- persistent tiles ----
            wqkv_sb = cp.tile([128, NC, 512], dt.bfloat16, tag="wqkv")
            wot_sb = cp.tile([128, NC, D], dt.bfloat16, tag="wot")
            cbar_sb = cp.tile([128, NST, EW], dt.bfloat16, tag="cbar")
            sbar_sb = cp.tile([128, NST, EW], dt.bfloat16, tag="sbar")
            tri_sb = cp.tile([128, KC], dt.bfloat16, tag="tri")
            es_sb = cp.tile([128, HL], dt.float32, tag="es")
            ones128 = cp.tile([128, 128], dt.bfloat16, tag="ones128")
            nc.vector.memset(ones128[:], 1.0)
            ident = cp.tile([128, 128], dt.bfloat16, tag="ident")
            make_identity(nc, ident[:])


            # q0|q1|k transposed: [dh, {q0,q1,k}, st, s]
            qkT = cp.tile([128, 3, NST, 128], dt.bfloat16, tag="qkT")
            v_sb = cp.tile([128, NST, DH], dt.bfloat16, tag="v")      # [s, st, dh]
            # gathered attention features per phase: [p, c, q] (c = global hd chunk)
            atT = [cp.tile([128, NC, 128], dt.bfloat16, tag=f"atT{p}", name=f"atT{p}")
                   for p in range(2)]

            # ---- AllToAll bounce buffers ----
            # phase a (q 0:1024): one 512 KB buffer carrying both heads.
            # phase b (q 1024:2048): split by head into two 256 KB halves so
            # the h0 half flies while h1's attention still computes.
            a2a_in0 = dp.tile([HL * 128 * N_CORES, 128], dt.bfloat16, name="a2a_in0")
            a2a_out0 = dp.tile([HL * 128 * N_CORES, 128], dt.bfloat16, name="a2a_out0")
            b_ins = [dp.tile([128 * N_CORES, 128], dt.bfloat16, name=f"b_in{h}")
                     for h in range(HL)]
            b_outs = [dp.tile([128 * N_CORES, 128], dt.bfloat16, name=f"b_out{h}")
                      for h in range(HL)]

            # ---- input DMA issue ----
            xts = xt.rearrange("p (st e) -> p st e", st=NST)
            cbr = cbar.rearrange("p (st e) -> p st e", st=NST)
            sbr = sbar.rearrange("p (st e) -> p st e", st=NST)
            wqr = wqkv.rearrange("p (c e) -> p c e", c=NC)
            xts4 = xt.rearrange("p (st c e) -> p st c e", st=NST, c=NC)

            xt_tiles = [xp.tile([128, NC, 128], dt.bfloat16, tag="xt", bufs=6,
                                name=f"xt{st}") for st in range(NST)]

            # critical first bytes: wqkv on sync+scalar, xt st0 in pieces so
            # proj can start on chunk 0 immediately
            nc.scalar.dma_start(xt_tiles[0][:, 0:2, :], xts4[:, 0, 0:2, :])
            nc.sync.dma_start(wqkv_sb[:, 0:4, :], wqr[:, 0:4, :])
            nc.scalar.dma_start(xt_tiles[0][:, 2:8, :], xts4[:, 0, 2:8, :])
            nc.sync.dma_start(wqkv_sb[:, 4:8, :], wqr[:, 4:8, :])
            nc.scalar.dma_start(xt_tiles[0][:, 8:16, :], xts4[:, 0, 8:16, :])
            nc.scalar.dma_start(wqkv_sb[:, 8:12, :], wqr[:, 8:12, :])
            nc.scalar.dma_start(wqkv_sb[:, 12:16, :], wqr[:, 12:16, :])
            nc.scalar.dma_start(cbar_sb[:, 0:4, :], cbr[:, 0:4, :])
            nc.scalar.dma_start(sbar_sb[:, 0:4, :], sbr[:, 0:4, :])
            nc.scalar.dma_start(es_sb[:], esd)
            nc.scalar.dma_start(tri_sb[:], trimask)
            # xt stream split across two queues (even tiles on sync, odd on
            # gpsimd), issued in consumption order — DMA starts are globally
            # ordered, so out-of-order issue + buffer WAR would deadlock.
            # 6-deep buffer ring keeps the prefetch well ahead. The later
            # rope-table groups interleave AFTER the early xt tiles so their
            # 2.25 MB doesn't starve the startup-critical stream.
            for st in range(1, NST):
                eng = nc.sync if st % 2 == 0 else nc.gpsimd
                eng.dma_start(xt_tiles[st][:], xts[:, st, :])
                if st in (3, 5, 7):
                    g = (st - 1) // 2
                    nc.gpsimd.dma_start(cbar_sb[:, 4 * g:4 * g + 4, :],
                                        cbr[:, 4 * g:4 * g + 4, :])
                    nc.gpsimd.dma_start(sbar_sb[:, 4 * g:4 * g + 4, :],
                                        sbr[:, 4 * g:4 * g + 4, :])
            # full woT issued lazily inside the st loop (gpsimd queue) so its
            # 8 MB doesn't compete with the startup-critical streams
            wor = wot.rearrange("p (c e) -> p c e", c=NC)

            # PE warm-up: junk matmuls while the first inputs stream in, so
            # the HAM clock gate reaches 8/8 before proj(0) issues.
            # (reuses the lacc PSUM slot, which is first needed much later)
            warm = pp.tile([128, QT], dt.float32, tag="lacc", bufs=1, name="warm")
            for _ in range(48):
                nc.tensor.matmul(warm[:, 0:128], ident[:], ones128[:],
                                 start=True, stop=True)

            def proj(st):
                mm = pp.tile([128, 512], dt.float32, tag="mm")  # q[0:256] | k[256:384] | v[384:512]
                for c in range(NC):
                    nc.tensor.matmul(mm[:], xt_tiles[st][:, c, :], wqkv_sb[:, c, :],
                                     start=(c == 0), stop=(c == NC - 1))

                # evacuate PSUM: q|k to f32 SBUF, v to bf16 (ACT Copy is
                # table-less, so these live on the scalar engine)
                qk = wp.tile([128, EW], dt.float32, tag="qk", bufs=4)
                nc.scalar.copy(qk[:], mm[:, 0:EW])
                nc.scalar.copy(v_sb[:, st, :], mm[:, EW:512])

                # qk-norm: ssq via fused square+accum on DVE
                ssq = wp.tile([128, 4], dt.float32, tag="ssq")
                scr = wp.tile([128, 128], dt.float32, tag="scr")
                for i in range(3):
                    nc.vector.scalar_tensor_tensor(
                        out=scr[:], in0=qk[:, i * DH:(i + 1) * DH], scalar=1.0,
                        in1=qk[:, i * DH:(i + 1) * DH], op0=AO.bypass, op1=AO.mult,
                        accum_out=ssq[:, i:i + 1])
                # rs = rsqrt(ssq), magic seed + 1 Newton step, all on DVE (a
                # Sqrt on ACT would evict the Exp activation table every st).
                # eps and the /DH + 1/sqrt(DH) score scales fold into the
                # qkhat constants below.
                nwi = wp.tile([128, 4], dt.int32, tag="nwi")
                nwt = wp.tile([128, 4], dt.float32, tag="nwt")
                rs = wp.tile([128, 4], dt.float32, tag="rs")
                nc.vector.tensor_scalar(out=nwi[:, 0:3],
                                        in0=ssq[:, 0:3].bitcast(dt.int32),
                                        scalar1=1, scalar2=None,
                                        op0=AO.logical_shift_right)
                # magic - (i>>1), via fp32 arith (±64 ulp noise is irrelevant
                # for a Newton seed); int32 out converts back value-wise
                nc.vector.tensor_scalar(out=nwi[:, 0:3], in0=nwi[:, 0:3],
                                        scalar1=-1.0, scalar2=1597463007.0,
                                        op0=AO.mult, op1=AO.add)
                y0 = nwi[:, 0:3].bitcast(dt.float32)
                nc.vector.tensor_tensor(out=nwt[:, 0:3], in0=y0, in1=y0, op=AO.mult)
                nc.vector.scalar_tensor_tensor(out=nwt[:, 0:3], in0=ssq[:, 0:3],
                                               scalar=-0.5, in1=nwt[:, 0:3],
                                               op0=AO.mult, op1=AO.mult)
                nc.vector.tensor_scalar(out=nwt[:, 0:3], in0=nwt[:, 0:3], scalar1=1.5,
                                        scalar2=None, op0=AO.add)
                nc.vector.tensor_tensor(out=rs[:, 0:3], in0=y0, in1=nwt[:, 0:3],
                                        op=AO.mult)

                # merged rope for q0|q1|k: u = qk*cos; w = pairswap(qk)*(+-sin)
                u1 = wp.tile([128, EW], dt.float32, tag="u1")
                w = wp.tile([128, EW], dt.float32, tag="w")
                nc.vector.tensor_tensor(out=u1[:], in0=qk[:], in1=cbar_sb[:, st, :],
                                        op=AO.mult)
                nc.vector.tensor_tensor(out=w[:, 0:EW:2], in0=qk[:, 1:EW:2],
                                        in1=sbar_sb[:, st, 0:EW:2], op=AO.mult)
                nc.vector.tensor_tensor(out=w[:, 1:EW:2], in0=qk[:, 0:EW:2],
                                        in1=sbar_sb[:, st, 1:EW:2], op=AO.mult)
                nc.vector.tensor_add(out=u1[:], in0=u1[:], in1=w[:])
                # qhat = u1 * rsqrt(ssq) * sqrt(DH)  (== u1 * rsqrt(ssq/DH));
                # khat = u1 * rsqrt(ssq)             (folds the 1/sqrt(DH) score scale)
                qkhat = wp.tile([128, EW], dt.bfloat16, tag="qkhat")
                for i in range(3):
                    if i < 2:
                        nc.vector.tensor_scalar(out=qkhat[:, i * DH:(i + 1) * DH],
                                                in0=u1[:, i * DH:(i + 1) * DH],
                                                scalar1=rs[:, i:i + 1],
                                                scalar2=float(np.sqrt(DH)),
                                                op0=AO.mult, op1=AO.mult)
                    else:
                        nc.vector.tensor_scalar(out=qkhat[:, i * DH:(i + 1) * DH],
                                                in0=u1[:, i * DH:(i + 1) * DH],
                                                scalar1=rs[:, i:i + 1], scalar2=None,
                                                op0=AO.mult)

                # PE transposes into one PSUM tile; single ACT evacuation
                tp = pp.tile([128, EW], dt.bfloat16, tag="tp")
                for i in range(3):
                    nc.tensor.transpose(tp[:, i * DH:(i + 1) * DH],
                                        qkhat[:, i * DH:(i + 1) * DH], ident[:])
                nc.scalar.copy(qkT[:, :, st, :], tp[:].rearrange("p (i e) -> p i e", i=3))

            def attn_group(t, head_done_cb=None):
                # chunk i: i < 4t -> full k-chunk c=i over q cols [0:512)
                #          i >= 4t -> diagonal chunk c=4t+j over q cols [128j:512)
                nch = 4 * t + 4

                def chunk_info(i):
                    if i < 4 * t:
                        return i, 0
                    j = i - 4 * t
                    return 4 * t + j, 128 * j

                # flat (h, i) task list with one-task software pipelining so
                # the exp of each chunk hides under the previous chunk's
                # accumulation matmuls, across head boundaries too
                accs = {}
                pts = {}

                def emit_score(h, i):
                    if i == 0:
                        lacc = pp.tile([128, QT], dt.float32, tag="lacc", bufs=1)
                        oacc = pp.tile([128, QT], dt.float32, tag="oacc", bufs=1)
                        accs[h] = (lacc, oacc)
                    c, qoff = chunk_info(i)
                    wd = QT - qoff
                    stp = pp.tile([128, QT], dt.float32, tag="stp")
                    nc.tensor.matmul(stp[:, 0:wd], qkT[:, 2, c, :],
                                     qkT[:, h, 4 * t + qoff // 128:4 * t + 4, :],
                                     start=True, stop=True)
                    pt = wp.tile([128, QT], dt.bfloat16, tag="pt", bufs=4)
                    nc.scalar.activation(pt[:, 0:wd], stp[:, 0:wd], AF.Exp)
                    if i >= 4 * t:
                        nc.gpsimd.tensor_tensor(out=pt[:, 0:KC], in0=pt[:, 0:KC],
                                                in1=tri_sb[:], op=AO.mult)
                    pts[(h, i)] = (pt, c, qoff, wd)

                def emit_acc(h, i):
                    pt, c, qoff, wd = pts.pop((h, i))
                    lacc, oacc = accs[h]
                    last = i == nch - 1
                    nc.tensor.matmul(lacc[:, qoff:QT], ones128[:], pt[:, 0:wd],
                                     start=(i == 0), stop=last)
                    nc.tensor.matmul(oacc[:, qoff:QT], v_sb[:, c, :], pt[:, 0:wd],
                                     start=(i == 0), stop=last)
                    if last:
                        emit_finish(h)

                def emit_finish(h):
                    # out = oacc / (lacc + exp(sink))
                    lacc, oacc = accs[h]
                    tmp = wp.tile([128, QT], dt.float32, tag="tmp")
                    nc.vector.tensor_scalar(out=tmp[:], in0=lacc[:],
                                            scalar1=es_sb[:, h:h + 1], scalar2=None,
                                            op0=AO.add)
                    rr = wp.tile([128, QT], dt.float32, tag="rr")
                    nc.vector.reciprocal_approx_fast(rr[:], tmp[:])
                    att = wp.tile([128, QT], dt.bfloat16, tag="att")
                    nc.vector.tensor_tensor(out=att[:], in0=oacc[:], in1=rr[:], op=AO.mult)
                    # scatter the 4 q-subtiles to their destination-rank slots
                    r0 = 4 * (t % 2)
                    if t < 2:
                        dst = a2a_in0[:].rearrange("(r h pp) q -> pp r h q",
                                                   r=N_CORES, h=HL)[:, r0:r0 + 4, h, :]
                    else:
                        dst = b_ins[h][:].rearrange("(r pp) q -> pp r q",
                                                    r=N_CORES)[:, r0:r0 + 4, :]
                    nc.scalar.dma_start(dst, att[:].rearrange("p (j q) -> p j q", j=4))

                tasks = [(h, i) for h in range(HL) for i in range(nch)]
                emit_score(*tasks[0])
                for j in range(1, len(tasks)):
                    emit_score(*tasks[j])
                    emit_acc(*tasks[j - 1])
                    if head_done_cb is not None and tasks[j - 1] == (0, nch - 1):
                        head_done_cb()
                emit_acc(*tasks[-1])

            def emit_a2a(ins_t, outs_t):
                nc.gpsimd.collective_compute(
                    "AllToAll", AO.bypass,
                    replica_groups=[list(range(N_CORES))],
                    ins=[ins_t[:].opt()], outs=[outs_t[:].opt()],
                )

            def load_atT0():
                # on sync (nothing latency-critical sits behind it there), in
                # 4 pieces so wo's first accumulation chunk starts ~2us sooner
                src = a2a_out0[:].rearrange("(g c pp) q -> pp g c q", pp=128, g=4)
                for g in range(4):
                    nc.sync.dma_start(atT[0][:, 4 * g:4 * g + 4, :], src[:, g, :, :])

            def load_atT1(h):
                # b_outs[h] chunk r = rank r's head h = global hd chunk 2r+h
                # (2 pieces so wo's accumulation starts on the first 4 chunks
                # while the rest land)
                src = b_outs[h][:].rearrange("(g c pp) q -> pp g c q", pp=128, g=2)
                for g in range(2):
                    nc.sync.dma_start(atT[1][:, h + 8 * g:8 * g + 8:2, :],
                                      src[:, g, :, :])

            def wo_phase(p):
                # y[q128, :] = sum_c atT[p][:, c, :].T @ woT[:, c, :]
                # 4 PSUM banks cover the full 2048 douts in one accumulation
                # sweep: one stationary load per chunk feeds 2048 moving cols.
                # phase 1 accumulates even hd-chunks (from the b1 collective)
                # first so they overlap the b2 half still in flight.
                order = (list(range(NC)) if p == 0 else
                         list(range(0, NC, 2)) + list(range(1, NC, 2)))
                if p == 0:
                    # one 4-bank sweep; its evacuation overlaps phase-1 MMs
                    yps = [pp.tile([128, 512], dt.float32, tag=tg, name=f"yp0_{k}")
                           for k, tg in enumerate(("mm", "mm", "stp", "stp"))]
                    for ci, c in enumerate(order):
                        for k in range(4):
                            nc.tensor.matmul(yps[k][:], atT[p][:, c, :],
                                             wot_sb[:, c, 512 * k:512 * (k + 1)],
                                             start=(ci == 0), stop=(ci == NC - 1))
                    ysb = wp.tile([128, 2048], dt.float32, tag="ysb", bufs=1)
                    for k in range(4):
                        nc.scalar.copy(ysb[:, 512 * k:512 * (k + 1)], yps[k][:])
                    nc.scalar.dma_start(y_out[0:128, 0:1024], ysb[:, 0:1024])
                    nc.scalar.dma_start(y_out[0:128, 1024:2048], ysb[:, 1024:2048])
                else:
                    # two 2-bank sweeps on distinct banks: sweep A's evac+DMA
                    # runs under sweep B's MMs, trimming the trailing chain
                    for sw, tg in enumerate(("mm", "stp")):
                        yp0 = pp.tile([128, 512], dt.float32, tag=tg, name=f"ypa{sw}")
                        yp1 = pp.tile([128, 512], dt.float32, tag=tg, name=f"ypb{sw}")
                        d0 = 1024 * sw
                        for ci, c in enumerate(order):
                            nc.tensor.matmul(yp0[:], atT[p][:, c, :],
                                             wot_sb[:, c, d0:d0 + 512],
                                             start=(ci == 0), stop=(ci == NC - 1))
                            nc.tensor.matmul(yp1[:], atT[p][:, c, :],
                                             wot_sb[:, c, d0 + 512:d0 + 1024],
                                             start=(ci == 0), stop=(ci == NC - 1))
                        ysb = wp.tile([128, 1024], dt.float32, tag="ysb2")
                        nc.scalar.copy(ysb[:, 0:512], yp0[:])
                        nc.scalar.copy(ysb[:, 512:1024], yp1[:])
                        nc.scalar.dma_start(y_out[128:256, d0:d0 + 1024], ysb[:])

            # ---- emission: all local attention first; wo (collective-
            # dependent) pinned last so the PE FIFO never stalls on a
            # collective while local work remains ----
            for st in range(NST):
                proj(st)
                if st < 4:
                    # keep the HAM clock gate warm across early xt-feed stalls
                    # (stops before attn_group(0) reuses the lacc PSUM slot)
                    for _ in range(6):
                        nc.tensor.matmul(warm[:, 0:128], ident[:], ones128[:],
                                         start=True, stop=True)
                if st in (1, 2):
                    g = st - 1
                    nc.gpsimd.dma_start(wot_sb[:, 4 * g:4 * g + 4, :],
                                        wor[:, 4 * g:4 * g + 4, :])
                if st >= 4 and st % 4 == 0:
                    attn_group(st // 4 - 1)
                    if st == 4:
                        for g in (2, 3):
                            nc.gpsimd.dma_start(wot_sb[:, 4 * g:4 * g + 4, :],
                                                wor[:, 4 * g:4 * g + 4, :])
                    if st == 8:
                        emit_a2a(a2a_in0, a2a_out0)
                    if st == 12:
                        # A2A-a certainly complete by the time the sync queue
                        # reaches this (it sits behind the whole xt stream)
                        load_atT0()
            attn_group(NQT - 1, head_done_cb=lambda: emit_a2a(b_ins[0], b_outs[0]))
            emit_a2a(b_ins[1], b_outs[1])
            with tc.tile_wait_until(1.0):
                load_atT1(0)
                wo_phase(0)
            with tc.tile_wait_until(1.1):
                load_atT1(1)
                wo_phase(1)

    nc.compile()
    return nc


def prep_inputs(x, freqs_cis, wq, wk, wv, wo, sinks):
    """Host-side sharding/layout prep. Returns in_maps for the 8 cores.

    All tensors are pre-tiled partition-major ([p, ...]) so DMAs move
    long contiguous per-partition runs.
    """
    x2 = np.ascontiguousarray(np.asarray(x, np.float32).reshape(S, D))
    xt = x2.T.astype(BF16)                                    # [D, S] = [(c p), (st s)]
    xt_h = np.ascontiguousarray(
        xt.reshape(NC, 128, NST, 128).transpose(1, 2, 0, 3).reshape(128, NST * NC * 128))

    fc = np.asarray(freqs_cis, np.float32)
    cos, sin = fc[:, :, 0], fc[:, :, 1]
    c1 = np.repeat(cos, 2, axis=1)             # [S, 128] pair-interleaved
    s1 = np.repeat(sin, 2, axis=1)
    cbar = np.concatenate([c1, c1, c1], axis=1).astype(np.float32)   # [S, 384] q0|q1|k
    sbar = np.concatenate([s1, s1, s1], axis=1).astype(np.float32)
    sbar[:, 0::2] *= -1.0                      # even outputs get -sin
    cbar_h = np.ascontiguousarray(
        cbar.reshape(NST, 128, EW).transpose(1, 0, 2).reshape(128, NST * EW)).astype(BF16)
    sbar_h = np.ascontiguousarray(
        sbar.reshape(NST, 128, EW).transpose(1, 0, 2).reshape(128, NST * EW)).astype(BF16)

    kr = np.arange(KC)[:, None]
    qr = np.arange(KC)[None, :]
    trimask = (qr >= kr).astype(np.float32).astype(BF16)      # [128, 128]

    wq = np.asarray(wq, np.float32)
    wk = np.asarray(wk, np.float32)
    wv = np.asarray(wv, np.float32)
    wo = np.asarray(wo, np.float32)
    sinks = np.asarray(sinks, np.float32)

    # full woT, identical on every core: [hd, dout] -> [p, c, dout]
    woT = np.ascontiguousarray(wo.T).astype(BF16)             # [HD=2048, D]
    wot_h = np.ascontiguousarray(
        woT.reshape(NC, 128, D).transpose(1, 0, 2).reshape(128, NC * D))

    in_maps = []
    for d in range(N_CORES):
        kv = d // 2
        es = np.exp(sinks[2 * d:2 * d + 2]).astype(np.float32)
        wqkv = np.concatenate([
            wq[d * 256:(d + 1) * 256, :].T,
            wk[kv * 128:(kv + 1) * 128, :].T,
            wv[kv * 128:(kv + 1) * 128, :].T,
        ], axis=1).astype(BF16)                               # [D, 512] = [(c p), e]
        wqkv_h = np.ascontiguousarray(
            wqkv.reshape(NC, 128, 512).transpose(1, 0, 2).reshape(128, NC * 512))
        in_maps.append({
            "xt": xt_h,
            "wqkv": wqkv_h,
            "wot": wot_h,
            "cbar": cbar_h,
            "sbar": sbar_h,
            "trimask": trimask,
            "es": np.repeat(es[None, :], 128, axis=0).astype(np.float32),
        })
    return in_maps


def assemble_output(results):
    """Interleave per-core q-row blocks: core d, phase p -> rows 1024p+128d."""
    y = np.zeros((S, D), dtype=np.float32)
    for d in range(N_CORES):
        yd = results[d]["y"]
        for p in range(2):
            y[1024 * p + 128 * d:1024 * p + 128 * d + 128, :] = yd[128 * p:128 * p + 128, :]
    return y.reshape(1, S, D)


_CACHED = {}


def kernel(x, freqs_cis, wq, wk, wv, wo, sinks):
    if "nc" not in _CACHED:
        _CACHED["nc"] = build()
    nc = _CACHED["nc"]
    in_maps = prep_inputs(x, freqs_cis, wq, wk, wv, wo, sinks)
    res = run_bass_kernel_spmd(nc, in_maps, list(range(N_CORES)), trace=False)
    return assemble_output(res.results)


# revision 43
# speedup vs baseline: 1.0397x; 1.0397x over previous
"""Distributed Bass kernel for nn_Attention_32701880992127 on 8 TRN2 NeuronCores.

Sharding (tensor parallel over heads): core d owns q-heads {2d, 2d+1} and
kv-head d//2 (GQA consecutive-repeat mapping). wq/wk/wv are column-split.

The output projection is ROW-split over q: instead of AllGathering the full
attention output to every core (8 MB of wire per core), each core sends core
j its q-slice via AllToAll (0.5 MB of wire), holds the FULL woT in SBUF, and
computes complete output rows y[q_slice, :]. Two AllToAll phases (q 0:1024 /
1024:2048) so phase-a's wo overlaps phase-b's attention; the host interleaves
the per-core row blocks back together.

All matmuls run in bf16 (f32 PSUM accumulation); elementwise math stays f32.
Softmax needs no max-subtraction (qk-norm bounds the scores); the sink
correction folds into the denominator:
    out_h = (sum_k exp(s_qk) v_k) / (exp(sink_h) + sum_k exp(s_qk)).
Scores are computed transposed (ST[k, q]) so exp's output directly feeds the
PV matmul as the moving operand. The causal diagonal 512-block is processed
as 4 k-chunks with shrinking q-windows; only the 128x128 diagonal block of
each chunk needs a (shared triangular) mask, applied on the GpSimd engine.

Engine discipline: the ACT engine runs ONLY Exp and Sqrt (its activation
table cache holds two functions; adding Copy caused a ~1.3us table reload per
proj tile). All PSUM evacuations run on DVE; the three per-st-tile PE
transposes (q0|q1|k) land in one PSUM tile and evacuate with a single DVE
copy. xt streams through a 4-deep buffer ring (frees 8 MB of SBUF for woT).
DMA issue is spread across the sync/scalar/vector/gpsimd queues.
"""
import numpy as np
import ml_dtypes

import concourse.mybir as mybir
import concourse.tile as tile
from concourse import bacc
from concourse.bass_utils import run_bass_kernel_spmd
from concourse.masks import make_identity

dt = mybir.dt
AO = mybir.AluOpType
AF = mybir.ActivationFunctionType
BF16 = ml_dtypes.bfloat16

N_CORES = 8
S = 2048            # sequence length
D = 2048            # model dim
DH = 128            # head dim
HL = 2              # local q heads per core
NC = 16             # d-chunks of 128
NST = 16            # s-tiles of 128
QT = 512            # attention q tile
NQT = S // QT
KC = 128            # attention k chunk
EW = 384            # rope width: q0|q1|k
RMS_EPS = 1.1920929e-07


def build():
    nc = bacc.Bacc("TRN2", target_bir_lowering=False, debug=False, num_devices=N_CORES)

    # all inputs pre-tiled partition-major on the host: [p, ...] with long
    # contiguous per-partition runs
    xt = nc.dram_tensor("xt", [128, NST * NC * 128], dt.bfloat16,
                        kind="ExternalInput").ap()            # [p, st, c, s]
    wqkv = nc.dram_tensor("wqkv", [128, NC * 512], dt.bfloat16,
                          kind="ExternalInput").ap()          # [p, c, e]
    wot = nc.dram_tensor("wot", [128, NC * D], dt.bfloat16,
                         kind="ExternalInput").ap()           # [p, c, dout] FULL woT
    cbar = nc.dram_tensor("cbar", [128, NST * EW], dt.bfloat16,
                          kind="ExternalInput").ap()          # [p, st, e] cos for q0|q1|k
    sbar = nc.dram_tensor("sbar", [128, NST * EW], dt.bfloat16,
                          kind="ExternalInput").ap()          # sign-folded sin
    trimask = nc.dram_tensor("trimask", [KC, KC], dt.bfloat16, kind="ExternalInput").ap()
    esd = nc.dram_tensor("es", [128, HL], dt.float32, kind="ExternalInput").ap()
    # output: 2 phases x 128 q rows, full model dim
    y_out = nc.dram_tensor("y", [2 * 128, D], dt.float32, kind="ExternalOutput").ap()

    with tile.TileContext(nc) as tc:
        with (
            tc.tile_pool(name="const", bufs=1) as cp,
            tc.tile_pool(name="xts", bufs=4) as xp,
            tc.tile_pool(name="work", bufs=2) as wp,
            tc.tile_pool(name="psum", bufs=2, space="PSUM") as pp,
            tc.tile_pool(name="dram", bufs=1, space="DRAM") as dp,
        ):
            # ---- persistent tiles ----
            wqkv_sb = cp.tile([128, NC, 512], dt.bfloat16, tag="wqkv")
            wot_sb = cp.tile([128, NC, D], dt.bfloat16, tag="wot")
            cbar_sb = cp.tile([128, NST, EW], dt.bfloat16, tag="cbar")
            sbar_sb = cp.tile([128, NST, EW], dt.bfloat16, tag="sbar")
            tri_sb = cp.tile([128, KC], dt.bfloat16, tag="tri")
            es_sb = cp.tile([128, HL], dt.float32, tag="es")
            ones128 = cp.tile([128, 128], dt.bfloat16, tag="ones128")
            nc.vector.memset(ones128[:], 1.0)
            ident = cp.tile([128, 128], dt.bfloat16, tag="ident")
            make_identity(nc, ident[:])


            # q0|q1|k transposed: [dh, {q0,q1,k}, st, s]
            qkT = cp.tile([128, 3, NST, 128], dt.bfloat16, tag="qkT")
            v_sb = cp.tile([128, NST, DH], dt.bfloat16, tag="v")      # [s, st, dh]
            # gathered attention features per phase: [p, c, q] (c = global hd chunk)
            atT = [cp.tile([128, NC, 128], dt.bfloat16, tag=f"atT{p}", name=f"atT{p}")
                   for p in range(2)]

            # ---- AllToAll bounce buffers ----
            # phase a (q 0:1024): one 512 KB buffer carrying both heads.
            # phase b (q 1024:2048): split by head into two 256 KB halves so
            # the h0 half flies while h1's attention still computes.
            a2a_in0 = dp.tile([HL * 128 * N_CORES, 128], dt.bfloat16, name="a2a_in0")
            a2a_out0 = dp.tile([HL * 128 * N_CORES, 128], dt.bfloat16, name="a2a_out0")
            b_ins = [dp.tile([128 * N_CORES, 128], dt.bfloat16, name=f"b_in{h}")
                     for h in range(HL)]
            b_outs = [dp.tile([128 * N_CORES, 128], dt.bfloat16, name=f"b_out{h}")
                      for h in range(HL)]

            # ---- input DMA issue ----
            xts = xt.rearrange("p (st e) -> p st e", st=NST)
            cbr = cbar.rearrange("p (st e) -> p st e", st=NST)
            sbr = sbar.rearrange("p (st e) -> p st e", st=NST)
            wqr = wqkv.rearrange("p (c e) -> p c e", c=NC)
            xts4 = xt.rearrange("p (st c e) -> p st c e", st=NST, c=NC)

            xt_tiles = [xp.tile([128, NC, 128], dt.bfloat16, tag="xt", bufs=6,
                                name=f"xt{st}") for st in range(NST)]

            # critical first bytes: wqkv on sync+scalar, xt st0 in pieces so
            # proj can start on chunk 0 immediately
            nc.scalar.dma_start(xt_tiles[0][:, 0:2, :], xts4[:, 0, 0:2, :])
            nc.sync.dma_start(wqkv_sb[:, 0:4, :], wqr[:, 0:4, :])
            nc.scalar.dma_start(xt_tiles[0][:, 2:8, :], xts4[:, 0, 2:8, :])
            nc.sync.dma_start(wqkv_sb[:, 4:8, :], wqr[:, 4:8, :])
            nc.scalar.dma_start(xt_tiles[0][:, 8:16, :], xts4[:, 0, 8:16, :])
            nc.scalar.dma_start(wqkv_sb[:, 8:12, :], wqr[:, 8:12, :])
            nc.scalar.dma_start(wqkv_sb[:, 12:16, :], wqr[:, 12:16, :])
            nc.scalar.dma_start(cbar_sb[:, 0:4, :], cbr[:, 0:4, :])
            nc.scalar.dma_start(sbar_sb[:, 0:4, :], sbr[:, 0:4, :])
            nc.scalar.dma_start(es_sb[:], esd)
            nc.scalar.dma_start(tri_sb[:], trimask)
            # xt stream split across two queues (even tiles on sync, odd on
            # gpsimd), issued in consumption order — DMA starts are globally
            # ordered, so out-of-order issue + buffer WAR would deadlock.
            # 6-deep buffer ring keeps the prefetch well ahead. The later
            # rope-table groups interleave AFTER the early xt tiles so their
            # 2.25 MB doesn't starve the startup-critical stream.
            for st in range(1, NST):
                eng = nc.sync if st % 2 == 0 else nc.gpsimd
                if st <= 5:
                    # halves, so proj(st) starts on the first 8 chunks while
                    # the rest of the tile is still in flight
                    eng.dma_start(xt_tiles[st][:, 0:8, :], xts4[:, st, 0:8, :])
                    eng.dma_start(xt_tiles[st][:, 8:16, :], xts4[:, st, 8:16, :])
                else:
                    eng.dma_start(xt_tiles[st][:], xts[:, st, :])
                if st in (3, 5, 7):
                    g = (st - 1) // 2
                    nc.gpsimd.dma_start(cbar_sb[:, 4 * g:4 * g + 4, :],
                                        cbr[:, 4 * g:4 * g + 4, :])
                    nc.gpsimd.dma_start(sbar_sb[:, 4 * g:4 * g + 4, :],
                                        sbr[:, 4 * g:4 * g + 4, :])
            # full woT issued lazily inside the st loop (gpsimd queue) so its
            # 8 MB doesn't compete with the startup-critical streams
            wor = wot.rearrange("p (c e) -> p c e", c=NC)

            # PE warm-up: junk matmuls while the first inputs stream in, so
            # the HAM clock gate reaches 8/8 before proj(0) issues.
            # (reuses the lacc PSUM slot, which is first needed much later)
            warm = pp.tile([128, QT], dt.float32, tag="lacc", bufs=1, name="warm")
            for _ in range(48):
                nc.tensor.matmul(warm[:, 0:128], ident[:], ones128[:],
                                 start=True, stop=True)

            def proj(st):
                mm = pp.tile([128, 512], dt.float32, tag="mm")  # q[0:256] | k[256:384] | v[384:512]
                for c in range(NC):
                    nc.tensor.matmul(mm[:], xt_tiles[st][:, c, :], wqkv_sb[:, c, :],
                                     start=(c == 0), stop=(c == NC - 1))

                # evacuate PSUM: q|k to f32 SBUF, v to bf16 (ACT Copy is
                # table-less, so these live on the scalar engine)
                qk = wp.tile([128, EW], dt.float32, tag="qk", bufs=4)
                nc.scalar.copy(qk[:], mm[:, 0:EW])
                nc.scalar.copy(v_sb[:, st, :], mm[:, EW:512])

                # qk-norm: ssq via fused square+accum on DVE
                ssq = wp.tile([128, 4], dt.float32, tag="ssq")
                scr = wp.tile([128, 128], dt.float32, tag="scr")
                for i in range(3):
                    nc.vector.scalar_tensor_tensor(
                        out=scr[:], in0=qk[:, i * DH:(i + 1) * DH], scalar=1.0,
                        in1=qk[:, i * DH:(i + 1) * DH], op0=AO.bypass, op1=AO.mult,
                        accum_out=ssq[:, i:i + 1])
                # rs = rsqrt(ssq), magic seed + 1 Newton step, all on DVE (a
                # Sqrt on ACT would evict the Exp activation table every st).
                # eps and the /DH + 1/sqrt(DH) score scales fold into the
                # qkhat constants below.
                nwi = wp.tile([128, 4], dt.int32, tag="nwi")
                nwt = wp.tile([128, 4], dt.float32, tag="nwt")
                rs = wp.tile([128, 4], dt.float32, tag="rs")
                nc.vector.tensor_scalar(out=nwi[:, 0:3],
                                        in0=ssq[:, 0:3].bitcast(dt.int32),
                                        scalar1=1, scalar2=None,
                                        op0=AO.logical_shift_right)
                # magic - (i>>1), via fp32 arith (±64 ulp noise is irrelevant
                # for a Newton seed); int32 out converts back value-wise
                nc.vector.tensor_scalar(out=nwi[:, 0:3], in0=nwi[:, 0:3],
                                        scalar1=-1.0, scalar2=1597463007.0,
                                        op0=AO.mult, op1=AO.add)
                y0 = nwi[:, 0:3].bitcast(dt.float32)
                nc.vector.tensor_tensor(out=nwt[:, 0:3], in0=y0, in1=y0, op=AO.mult)
                nc.vector.scalar_tensor_tensor(out=nwt[:, 0:3], in0=ssq[:, 0:3],
                                               scalar=-0.5, in1=nwt[:, 0:3],
                                               op0=AO.mult, op1=AO.mult)
                nc.vector.tensor_scalar(out=nwt[:, 0:3], in0=nwt[:, 0:3], scalar1=1.5,
                                        scalar2=None, op0=AO.add)
                nc.vector.tensor_tensor(out=rs[:, 0:3], in0=y0, in1=nwt[:, 0:3],
                                        op=AO.mult)

                # merged rope for q0|q1|k: u = qk*cos; w = pairswap(qk)*(+-sin)
                u1 = wp.tile([128, EW], dt.float32, tag="u1")
                w = wp.tile([128, EW], dt.float32, tag="w")
                nc.vector.tensor_tensor(out=u1[:], in0=qk[:], in1=cbar_sb[:, st, :],
                                        op=AO.mult)
                nc.vector.tensor_tensor(out=w[:, 0:EW:2], in0=qk[:, 1:EW:2],
                                        in1=sbar_sb[:, st, 0:EW:2], op=AO.mult)
                nc.vector.tensor_tensor(out=w[:, 1:EW:2], in0=qk[:, 0:EW:2],
                                        in1=sbar_sb[:, st, 1:EW:2], op=AO.mult)
                nc.vector.tensor_add(out=u1[:], in0=u1[:], in1=w[:])
                # qhat = u1 * rsqrt(ssq) * sqrt(DH)  (== u1 * rsqrt(ssq/DH));
                # khat = u1 * rsqrt(ssq)             (folds the 1/sqrt(DH) score scale)
                qkhat = wp.tile([128, EW], dt.bfloat16, tag="qkhat")
                for i in range(3):
                    if i < 2:
                        nc.vector.tensor_scalar(out=qkhat[:, i * DH:(i + 1) * DH],
                                                in0=u1[:, i * DH:(i + 1) * DH],
                                                scalar1=rs[:, i:i + 1],
                                                scalar2=float(np.sqrt(DH)),
                                                op0=AO.mult, op1=AO.mult)
                    else:
                        nc.vector.tensor_scalar(out=qkhat[:, i * DH:(i + 1) * DH],
                                                in0=u1[:, i * DH:(i + 1) * DH],
                                                scalar1=rs[:, i:i + 1], scalar2=None,
                                                op0=AO.mult)

                # PE transposes into one PSUM tile; single ACT evacuation
                tp = pp.tile([128, EW], dt.bfloat16, tag="tp")
                for i in range(3):
                    nc.tensor.transpose(tp[:, i * DH:(i + 1) * DH],
                                        qkhat[:, i * DH:(i + 1) * DH], ident[:])
                nc.scalar.copy(qkT[:, :, st, :], tp[:].rearrange("p (i e) -> p i e", i=3))

            def attn_group(t, head_done_cb=None):
                # chunk i: i < 4t -> full k-chunk c=i over q cols [0:512)
                #          i >= 4t -> diagonal chunk c=4t+j over q cols [128j:512)
                nch = 4 * t + 4

                def chunk_info(i):
                    if i < 4 * t:
                        return i, 0
                    j = i - 4 * t
                    return 4 * t + j, 128 * j

                # flat (h, i) task list with one-task software pipelining so
                # the exp of each chunk hides under the previous chunk's
                # accumulation matmuls, across head boundaries too
                accs = {}
                pts = {}

                def emit_score(h, i):
                    if i == 0:
                        lacc = pp.tile([128, QT], dt.float32, tag="lacc", bufs=1)
                        oacc = pp.tile([128, QT], dt.float32, tag="oacc", bufs=1)
                        accs[h] = (lacc, oacc)
                    c, qoff = chunk_info(i)
                    wd = QT - qoff
                    stp = pp.tile([128, QT], dt.float32, tag="stp")
                    nc.tensor.matmul(stp[:, 0:wd], qkT[:, 2, c, :],
                                     qkT[:, h, 4 * t + qoff // 128:4 * t + 4, :],
                                     start=True, stop=True)
                    pt = wp.tile([128, QT], dt.bfloat16, tag="pt", bufs=4)
                    nc.scalar.activation(pt[:, 0:wd], stp[:, 0:wd], AF.Exp)
                    if i >= 4 * t:
                        nc.gpsimd.tensor_tensor(out=pt[:, 0:KC], in0=pt[:, 0:KC],
                                                in1=tri_sb[:], op=AO.mult)
                    pts[(h, i)] = (pt, c, qoff, wd)

                def emit_acc(h, i):
                    pt, c, qoff, wd = pts.pop((h, i))
                    lacc, oacc = accs[h]
                    last = i == nch - 1
                    nc.tensor.matmul(lacc[:, qoff:QT], ones128[:], pt[:, 0:wd],
                                     start=(i == 0), stop=last)
                    nc.tensor.matmul(oacc[:, qoff:QT], v_sb[:, c, :], pt[:, 0:wd],
                                     start=(i == 0), stop=last)
                    if last:
                        emit_finish(h)

                def emit_finish(h):
                    # out = oacc / (lacc + exp(sink))
                    lacc, oacc = accs[h]
                    tmp = wp.tile([128, QT], dt.float32, tag="tmp")
                    nc.vector.tensor_scalar(out=tmp[:], in0=lacc[:],
                                            scalar1=es_sb[:, h:h + 1], scalar2=None,
                                            op0=AO.add)
                    rr = wp.tile([128, QT], dt.float32, tag="rr")
                    nc.vector.reciprocal_approx_fast(rr[:], tmp[:])
                    att = wp.tile([128, QT], dt.bfloat16, tag="att")
                    nc.vector.tensor_tensor(out=att[:], in0=oacc[:], in1=rr[:], op=AO.mult)
                    # scatter the 4 q-subtiles to their destination-rank slots
                    r0 = 4 * (t % 2)
                    if t < 2:
                        dst = a2a_in0[:].rearrange("(r h pp) q -> pp r h q",
                                                   r=N_CORES, h=HL)[:, r0:r0 + 4, h, :]
                    else:
                        dst = b_ins[h][:].rearrange("(r pp) q -> pp r q",
                                                    r=N_CORES)[:, r0:r0 + 4, :]
                    nc.scalar.dma_start(dst, att[:].rearrange("p (j q) -> p j q", j=4))

                tasks = [(h, i) for h in range(HL) for i in range(nch)]
                emit_score(*tasks[0])
                for j in range(1, len(tasks)):
                    emit_score(*tasks[j])
                    emit_acc(*tasks[j - 1])
                    if head_done_cb is not None and tasks[j - 1] == (0, nch - 1):
                        head_done_cb()
                emit_acc(*tasks[-1])

            def emit_a2a(ins_t, outs_t):
                nc.gpsimd.collective_compute(
                    "AllToAll", AO.bypass,
                    replica_groups=[list(range(N_CORES))],
                    ins=[ins_t[:].opt()], outs=[outs_t[:].opt()],
                )

            def load_atT0():
                # on sync (nothing latency-critical sits behind it there), in
                # 4 pieces so wo's first accumulation chunk starts ~2us sooner
                src = a2a_out0[:].rearrange("(g c pp) q -> pp g c q", pp=128, g=4)
                for g in range(4):
                    nc.sync.dma_start(atT[0][:, 4 * g:4 * g + 4, :], src[:, g, :, :])

            def load_atT1(h):
                # b_outs[h] chunk r = rank r's head h = global hd chunk 2r+h
                # (2 pieces so wo's accumulation starts on the first 4 chunks
                # while the rest land)
                src = b_outs[h][:].rearrange("(g c pp) q -> pp g c q", pp=128, g=2)
                for g in range(2):
                    nc.sync.dma_start(atT[1][:, h + 8 * g:8 * g + 8:2, :],
                                      src[:, g, :, :])

            def wo_phase(p):
                # y[q128, :] = sum_c atT[p][:, c, :].T @ woT[:, c, :]
                # 4 PSUM banks cover the full 2048 douts in one accumulation
                # sweep: one stationary load per chunk feeds 2048 moving cols.
                # phase 1 accumulates even hd-chunks (from the b1 collective)
                # first so they overlap the b2 half still in flight.
                order = (list(range(NC)) if p == 0 else
                         list(range(0, NC, 2)) + list(range(1, NC, 2)))
                if p == 0:
                    # one 4-bank sweep; its evacuation overlaps phase-1 MMs
                    yps = [pp.tile([128, 512], dt.float32, tag=tg, name=f"yp0_{k}")
                           for k, tg in enumerate(("mm", "mm", "stp", "stp"))]
                    for ci, c in enumerate(order):
                        for k in range(4):
                            nc.tensor.matmul(yps[k][:], atT[p][:, c, :],
                                             wot_sb[:, c, 512 * k:512 * (k + 1)],
                                             start=(ci == 0), stop=(ci == NC - 1))
                    ysb = wp.tile([128, 2048], dt.float32, tag="ysb", bufs=1)
                    for k in range(4):
                        nc.scalar.copy(ysb[:, 512 * k:512 * (k + 1)], yps[k][:])
                    nc.scalar.dma_start(y_out[0:128, 0:1024], ysb[:, 0:1024])
                    nc.scalar.dma_start(y_out[0:128, 1024:2048], ysb[:, 1024:2048])
                else:
                    # 4 banks again, but emitted ALL-EVENS-first across both
                    # bank pairs (so the full even workload precedes the b2
                    # wait in the PE FIFO), and evacuated as two halves so the
                    # first evac+DMA runs under the second half's final MMs
                    yps = [pp.tile([128, 512], dt.float32, tag=tg, name=f"yp1_{k}")
                           for k, tg in enumerate(("mm", "mm", "stp", "stp"))]
                    half = NC // 2
                    for part in range(2):
                        sub = order[part * half:(part + 1) * half]
                        for ci, c in enumerate(sub):
                            for k in range(4):
                                nc.tensor.matmul(
                                    yps[k][:], atT[p][:, c, :],
                                    wot_sb[:, c, 512 * k:512 * (k + 1)],
                                    start=(part == 0 and ci == 0),
                                    stop=(part == 1 and ci == half - 1))
                    ysb = wp.tile([128, 2048], dt.float32, tag="ysb", bufs=1)
                    for k in range(4):
                        nc.scalar.copy(ysb[:, 512 * k:512 * (k + 1)], yps[k][:])
                    nc.scalar.dma_start(y_out[128:256, 0:1024], ysb[:, 0:1024])
                    nc.scalar.dma_start(y_out[128:256, 1024:2048], ysb[:, 1024:2048])

            # ---- emission: all local attention first; wo (collective-
            # dependent) pinned last so the PE FIFO never stalls on a
            # collective while local work remains ----
            for st in range(NST):
                proj(st)
                if st in (1, 2):
                    g = st - 1
                    nc.gpsimd.dma_start(wot_sb[:, 4 * g:4 * g + 4, :],
                                        wor[:, 4 * g:4 * g + 4, :])
                if st >= 4 and st % 4 == 0:
                    attn_group(st // 4 - 1)
                    if st == 4:
                        for g in (2, 3):
                            nc.gpsimd.dma_start(wot_sb[:, 4 * g:4 * g + 4, :],
                                                wor[:, 4 * g:4 * g + 4, :])
                    if st == 8:
                        emit_a2a(a2a_in0, a2a_out0)
                    if st == 12:
                        # A2A-a certainly complete by the time the sync queue
                        # reaches this (it sits behind the whole xt stream)
                        load_atT0()
            attn_group(NQT - 1, head_done_cb=lambda: emit_a2a(b_ins[0], b_outs[0]))
            emit_a2a(b_ins[1], b_outs[1])
            with tc.tile_wait_until(1.0):
                load_atT1(0)
                wo_phase(0)
            with tc.tile_wait_until(1.1):
                load_atT1(1)
                wo_phase(1)

    nc.compile()
    return nc


def prep_inputs(x, freqs_cis, wq, wk, wv, wo, sinks):
    """Host-side sharding/layout prep. Returns in_maps for the 8 cores.

    All tensors are pre-tiled partition-major ([p, ...]) so DMAs move
    long contiguous per-partition runs.
    """
    x2 = np.ascontiguousarray(np.asarray(x, np.float32).reshape(S, D))
    xt = x2.T.astype(BF16)                                    # [D, S] = [(c p), (st s)]
    xt_h = np.ascontiguousarray(
        xt.reshape(NC, 128, NST, 128).transpose(1, 2, 0, 3).reshape(128, NST * NC * 128))

    fc = np.asarray(freqs_cis, np.float32)
    cos, sin = fc[:, :, 0], fc[:, :, 1]
    c1 = np.repeat(cos, 2, axis=1)             # [S, 128] pair-interleaved
    s1 = np.repeat(sin, 2, axis=1)
    cbar = np.concatenate([c1, c1, c1], axis=1).astype(np.float32)   # [S, 384] q0|q1|k
    sbar = np.concatenate([s1, s1, s1], axis=1).astype(np.float32)
    sbar[:, 0::2] *= -1.0                      # even outputs get -sin
    cbar_h = np.ascontiguousarray(
        cbar.reshape(NST, 128, EW).transpose(1, 0, 2).reshape(128, NST * EW)).astype(BF16)
    sbar_h = np.ascontiguousarray(
        sbar.reshape(NST, 128, EW).transpose(1, 0, 2).reshape(128, NST * EW)).astype(BF16)

    kr = np.arange(KC)[:, None]
    qr = np.arange(KC)[None, :]
    trimask = (qr >= kr).astype(np.float32).astype(BF16)      # [128, 128]

    wq = np.asarray(wq, np.float32)
    wk = np.asarray(wk, np.float32)
    wv = np.asarray(wv, np.float32)
    wo = np.asarray(wo, np.float32)
    sinks = np.asarray(sinks, np.float32)

    # full woT, identical on every core: [hd, dout] -> [p, c, dout]
    woT = np.ascontiguousarray(wo.T).astype(BF16)             # [HD=2048, D]
    wot_h = np.ascontiguousarray(
        woT.reshape(NC, 128, D).transpose(1, 0, 2).reshape(128, NC * D))

    in_maps = []
    for d in range(N_CORES):
        kv = d // 2
        es = np.exp(sinks[2 * d:2 * d + 2]).astype(np.float32)
        wqkv = np.concatenate([
            wq[d * 256:(d + 1) * 256, :].T,
            wk[kv * 128:(kv + 1) * 128, :].T,
            wv[kv * 128:(kv + 1) * 128, :].T,
        ], axis=1).astype(BF16)                               # [D, 512] = [(c p), e]
        wqkv_h = np.ascontiguousarray(
            wqkv.reshape(NC, 128, 512).transpose(1, 0, 2).reshape(128, NC * 512))
        in_maps.append({
            "xt": xt_h,
            "wqkv": wqkv_h,
            "wot": wot_h,
            "cbar": cbar_h,
            "sbar": sbar_h,
            "trimask": trimask,
            "es": np.repeat(es[None, :], 128, axis=0).astype(np.float32),
        })
    return in_maps


def assemble_output(results):
    """Interleave per-core q-row blocks: core d, phase p -> rows 1024p+128d."""
    y = np.zeros((S, D), dtype=np.float32)
    for d in range(N_CORES):
        yd = results[d]["y"]
        for p in range(2):
            y[1024 * p + 128 * d:1024 * p + 128 * d + 128, :] = yd[128 * p:128 * p + 128, :]
    return y.reshape(1, S, D)


_CACHED = {}


def kernel(x, freqs_cis, wq, wk, wv, wo, sinks):
    if "nc" not in _CACHED:
        _CACHED["nc"] = build()
    nc = _CACHED["nc"]
    in_maps = prep_inputs(x, freqs_cis, wq, wk, wv, wo, sinks)
    res = run_bass_kernel_spmd(nc, in_maps, list(range(N_CORES)), trace=False)
    return assemble_output(res.results)


# revision 48
# speedup vs baseline: 1.0444x; 1.0045x over previous
"""Distributed Bass kernel for nn_Attention_32701880992127 on 8 TRN2 NeuronCores.

Sharding (tensor parallel over heads): core d owns q-heads {2d, 2d+1} and
kv-head d//2 (GQA consecutive-repeat mapping). wq/wk/wv are column-split.

The output projection is ROW-split over q: instead of AllGathering the full
attention output to every core (8 MB of wire per core), each core sends core
j its q-slice via AllToAll (0.5 MB of wire), holds the FULL woT in SBUF, and
computes complete output rows y[q_slice, :]. Two AllToAll phases (q 0:1024 /
1024:2048) so phase-a's wo overlaps phase-b's attention; the host interleaves
the per-core row blocks back together.

All matmuls run in bf16 (f32 PSUM accumulation); elementwise math stays f32.
Softmax needs no max-subtraction (qk-norm bounds the scores); the sink
correction folds into the denominator:
    out_h = (sum_k exp(s_qk) v_k) / (exp(sink_h) + sum_k exp(s_qk)).
Scores are computed transposed (ST[k, q]) so exp's output directly feeds the
PV matmul as the moving operand. The causal diagonal 512-block is processed
as 4 k-chunks with shrinking q-windows; only the 128x128 diagonal block of
each chunk needs a (shared triangular) mask, applied on the GpSimd engine.

Engine discipline: the ACT engine runs ONLY Exp and Sqrt (its activation
table cache holds two functions; adding Copy caused a ~1.3us table reload per
proj tile). All PSUM evacuations run on DVE; the three per-st-tile PE
transposes (q0|q1|k) land in one PSUM tile and evacuate with a single DVE
copy. xt streams through a 4-deep buffer ring (frees 8 MB of SBUF for woT).
DMA issue is spread across the sync/scalar/vector/gpsimd queues.
"""
import numpy as np
import ml_dtypes

import concourse.mybir as mybir
import concourse.tile as tile
from concourse import bacc
from concourse.bass_utils import run_bass_kernel_spmd
from concourse.masks import make_identity

dt = mybir.dt
AO = mybir.AluOpType
AF = mybir.ActivationFunctionType
BF16 = ml_dtypes.bfloat16

N_CORES = 8
S = 2048            # sequence length
D = 2048            # model dim
DH = 128            # head dim
HL = 2              # local q heads per core
NC = 16             # d-chunks of 128
NST = 16            # s-tiles of 128
QT = 512            # attention q tile
NQT = S // QT
KC = 128            # attention k chunk
EW = 384            # rope width: q0|q1|k
RMS_EPS = 1.1920929e-07


def build():
    nc = bacc.Bacc("TRN2", target_bir_lowering=False, debug=False, num_devices=N_CORES)

    # all inputs pre-tiled partition-major on the host: [p, ...] with long
    # contiguous per-partition runs
    xt = nc.dram_tensor("xt", [128, NST * NC * 128], dt.bfloat16,
                        kind="ExternalInput").ap()            # [p, st, c, s]
    wqkv = nc.dram_tensor("wqkv", [128, NC * 512], dt.bfloat16,
                          kind="ExternalInput").ap()          # [p, c, e]
    wot = nc.dram_tensor("wot", [128, NC * D], dt.bfloat16,
                         kind="ExternalInput").ap()           # [p, c, dout] FULL woT
    cbar = nc.dram_tensor("cbar", [128, NST * EW], dt.bfloat16,
                          kind="ExternalInput").ap()          # [p, st, e] cos for q0|q1|k
    sbar = nc.dram_tensor("sbar", [128, NST * EW], dt.bfloat16,
                          kind="ExternalInput").ap()          # sign-folded sin
    trimask = nc.dram_tensor("trimask", [KC, KC], dt.bfloat16, kind="ExternalInput").ap()
    esd = nc.dram_tensor("es", [128, HL], dt.float32, kind="ExternalInput").ap()
    # output: 2 phases x 128 q rows, full model dim
    y_out = nc.dram_tensor("y", [2 * 128, D], dt.float32, kind="ExternalOutput").ap()

    with tile.TileContext(nc) as tc:
        with (
            tc.tile_pool(name="const", bufs=1) as cp,
            tc.tile_pool(name="xts", bufs=4) as xp,
            tc.tile_pool(name="work", bufs=2) as wp,
            tc.tile_pool(name="psum", bufs=2, space="PSUM") as pp,
            tc.tile_pool(name="dram", bufs=1, space="DRAM") as dp,
        ):
            # ---- persistent tiles ----
            wqkv_sb = cp.tile([128, NC, 512], dt.bfloat16, tag="wqkv")
            wot_sb = cp.tile([128, NC, D], dt.bfloat16, tag="wot")
            cbar_sb = cp.tile([128, NST, EW], dt.bfloat16, tag="cbar")
            sbar_sb = cp.tile([128, NST, EW], dt.bfloat16, tag="sbar")
            tri_sb = cp.tile([128, KC], dt.bfloat16, tag="tri")
            es_sb = cp.tile([128, HL], dt.float32, tag="es")
            ones128 = cp.tile([128, 128], dt.bfloat16, tag="ones128")
            nc.vector.memset(ones128[:], 1.0)
            ident = cp.tile([128, 128], dt.bfloat16, tag="ident")
            make_identity(nc, ident[:])


            # q0|q1|k transposed: [dh, {q0,q1,k}, st, s]
            qkT = cp.tile([128, 3, NST, 128], dt.bfloat16, tag="qkT")
            v_sb = cp.tile([128, NST, DH], dt.bfloat16, tag="v")      # [s, st, dh]
            # gathered attention features per phase: [p, c, q] (c = global hd chunk)
            atT = [cp.tile([128, NC, 128], dt.bfloat16, tag=f"atT{p}", name=f"atT{p}")
                   for p in range(2)]

            # ---- AllToAll bounce buffers ----
            # phase a (q 0:1024): one 512 KB buffer carrying both heads.
            # phase b (q 1024:2048): split by head into two 256 KB halves so
            # the h0 half flies while h1's attention still computes.
            a2a_in0 = dp.tile([HL * 128 * N_CORES, 128], dt.bfloat16, name="a2a_in0")
            a2a_out0 = dp.tile([HL * 128 * N_CORES, 128], dt.bfloat16, name="a2a_out0")
            b_ins = [dp.tile([128 * N_CORES, 128], dt.bfloat16, name=f"b_in{h}")
                     for h in range(HL)]
            b_outs = [dp.tile([128 * N_CORES, 128], dt.bfloat16, name=f"b_out{h}")
                      for h in range(HL)]

            # ---- input DMA issue ----
            xts = xt.rearrange("p (st e) -> p st e", st=NST)
            cbr = cbar.rearrange("p (st e) -> p st e", st=NST)
            sbr = sbar.rearrange("p (st e) -> p st e", st=NST)
            wqr = wqkv.rearrange("p (c e) -> p c e", c=NC)
            xts4 = xt.rearrange("p (st c e) -> p st c e", st=NST, c=NC)

            xt_tiles = [xp.tile([128, NC, 128], dt.bfloat16, tag="xt", bufs=6,
                                name=f"xt{st}") for st in range(NST)]

            # critical first bytes: wqkv on sync+scalar, xt st0 in pieces so
            # proj can start on chunk 0 immediately
            nc.scalar.dma_start(xt_tiles[0][:, 0:2, :], xts4[:, 0, 0:2, :])
            nc.sync.dma_start(wqkv_sb[:, 0:4, :], wqr[:, 0:4, :])
            nc.scalar.dma_start(xt_tiles[0][:, 2:8, :], xts4[:, 0, 2:8, :])
            nc.sync.dma_start(wqkv_sb[:, 4:8, :], wqr[:, 4:8, :])
            nc.scalar.dma_start(xt_tiles[0][:, 8:16, :], xts4[:, 0, 8:16, :])
            nc.scalar.dma_start(wqkv_sb[:, 8:12, :], wqr[:, 8:12, :])
            nc.scalar.dma_start(wqkv_sb[:, 12:16, :], wqr[:, 12:16, :])
            nc.scalar.dma_start(cbar_sb[:, 0:4, :], cbr[:, 0:4, :])
            nc.scalar.dma_start(sbar_sb[:, 0:4, :], sbr[:, 0:4, :])
            nc.scalar.dma_start(es_sb[:], esd)
            nc.scalar.dma_start(tri_sb[:], trimask)
            # xt stream split across two queues (even tiles on sync, odd on
            # gpsimd), issued in consumption order — DMA starts are globally
            # ordered, so out-of-order issue + buffer WAR would deadlock.
            # 6-deep buffer ring keeps the prefetch well ahead. The later
            # rope-table groups interleave AFTER the early xt tiles so their
            # 2.25 MB doesn't starve the startup-critical stream.
            for st in range(1, NST):
                eng = nc.sync if st % 2 == 0 else nc.gpsimd
                if st <= 5:
                    # halves, so proj(st) starts on the first 8 chunks while
                    # the rest of the tile is still in flight
                    eng.dma_start(xt_tiles[st][:, 0:8, :], xts4[:, st, 0:8, :])
                    eng.dma_start(xt_tiles[st][:, 8:16, :], xts4[:, st, 8:16, :])
                else:
                    eng.dma_start(xt_tiles[st][:], xts[:, st, :])
                if st in (3, 5, 7):
                    g = (st - 1) // 2
                    nc.gpsimd.dma_start(cbar_sb[:, 4 * g:4 * g + 4, :],
                                        cbr[:, 4 * g:4 * g + 4, :])
                    nc.gpsimd.dma_start(sbar_sb[:, 4 * g:4 * g + 4, :],
                                        sbr[:, 4 * g:4 * g + 4, :])
            # full woT issued lazily inside the st loop (gpsimd queue) so its
            # 8 MB doesn't compete with the startup-critical streams
            wor = wot.rearrange("p (c e) -> p c e", c=NC)

            # PE warm-up: junk matmuls while the first inputs stream in, so
            # the HAM clock gate reaches 8/8 before proj(0) issues.
            # (reuses the lacc PSUM slot, which is first needed much later)
            warm = pp.tile([128, QT], dt.float32, tag="lacc", bufs=1, name="warm")
            for _ in range(48):
                nc.tensor.matmul(warm[:, 0:128], ident[:], ones128[:],
                                 start=True, stop=True)

            def proj(st):
                mm = pp.tile([128, 512], dt.float32, tag="mm")  # q[0:256] | k[256:384] | v[384:512]
                for c in range(NC):
                    nc.tensor.matmul(mm[:], xt_tiles[st][:, c, :], wqkv_sb[:, c, :],
                                     start=(c == 0), stop=(c == NC - 1))

                # evacuate PSUM: q|k to f32 SBUF, v to bf16 (ACT Copy is
                # table-less, so these live on the scalar engine)
                qk = wp.tile([128, EW], dt.float32, tag="qk", bufs=4)
                nc.scalar.copy(qk[:], mm[:, 0:EW])
                nc.scalar.copy(v_sb[:, st, :], mm[:, EW:512])

                # qk-norm: ssq via fused square+accum on DVE
                ssq = wp.tile([128, 4], dt.float32, tag="ssq")
                scr = wp.tile([128, 128], dt.float32, tag="scr")
                for i in range(3):
                    nc.vector.scalar_tensor_tensor(
                        out=scr[:], in0=qk[:, i * DH:(i + 1) * DH], scalar=1.0,
                        in1=qk[:, i * DH:(i + 1) * DH], op0=AO.bypass, op1=AO.mult,
                        accum_out=ssq[:, i:i + 1])
                # rs = rsqrt(ssq), magic seed + 1 Newton step, all on DVE (a
                # Sqrt on ACT would evict the Exp activation table every st).
                # eps and the /DH + 1/sqrt(DH) score scales fold into the
                # qkhat constants below.
                nwi = wp.tile([128, 4], dt.int32, tag="nwi")
                nwt = wp.tile([128, 4], dt.float32, tag="nwt")
                rs = wp.tile([128, 4], dt.float32, tag="rs")
                nc.vector.tensor_scalar(out=nwi[:, 0:3],
                                        in0=ssq[:, 0:3].bitcast(dt.int32),
                                        scalar1=1, scalar2=None,
                                        op0=AO.logical_shift_right)
                # magic - (i>>1), via fp32 arith (±64 ulp noise is irrelevant
                # for a Newton seed); int32 out converts back value-wise
                nc.vector.tensor_scalar(out=nwi[:, 0:3], in0=nwi[:, 0:3],
                                        scalar1=-1.0, scalar2=1597463007.0,
                                        op0=AO.mult, op1=AO.add)
                y0 = nwi[:, 0:3].bitcast(dt.float32)
                nc.vector.tensor_tensor(out=nwt[:, 0:3], in0=y0, in1=y0, op=AO.mult)
                nc.vector.scalar_tensor_tensor(out=nwt[:, 0:3], in0=ssq[:, 0:3],
                                               scalar=-0.5, in1=nwt[:, 0:3],
                                               op0=AO.mult, op1=AO.mult)
                nc.vector.tensor_scalar(out=nwt[:, 0:3], in0=nwt[:, 0:3], scalar1=1.5,
                                        scalar2=None, op0=AO.add)
                nc.vector.tensor_tensor(out=rs[:, 0:3], in0=y0, in1=nwt[:, 0:3],
                                        op=AO.mult)

                # merged rope for q0|q1|k: u = qk*cos; w = pairswap(qk)*(+-sin)
                u1 = wp.tile([128, EW], dt.float32, tag="u1")
                w = wp.tile([128, EW], dt.float32, tag="w")
                nc.vector.tensor_tensor(out=u1[:], in0=qk[:], in1=cbar_sb[:, st, :],
                                        op=AO.mult)
                nc.vector.tensor_tensor(out=w[:, 0:EW:2], in0=qk[:, 1:EW:2],
                                        in1=sbar_sb[:, st, 0:EW:2], op=AO.mult)
                nc.vector.tensor_tensor(out=w[:, 1:EW:2], in0=qk[:, 0:EW:2],
                                        in1=sbar_sb[:, st, 1:EW:2], op=AO.mult)
                nc.vector.tensor_add(out=u1[:], in0=u1[:], in1=w[:])
                # qhat = u1 * rsqrt(ssq) * sqrt(DH)  (== u1 * rsqrt(ssq/DH));
                # khat = u1 * rsqrt(ssq)             (folds the 1/sqrt(DH) score scale)
                qkhat = wp.tile([128, EW], dt.bfloat16, tag="qkhat")
                for i in range(3):
                    if i < 2:
                        nc.vector.tensor_scalar(out=qkhat[:, i * DH:(i + 1) * DH],
                                                in0=u1[:, i * DH:(i + 1) * DH],
                                                scalar1=rs[:, i:i + 1],
                                                scalar2=float(np.sqrt(DH)),
                                                op0=AO.mult, op1=AO.mult)
                    else:
                        nc.vector.tensor_scalar(out=qkhat[:, i * DH:(i + 1) * DH],
                                                in0=u1[:, i * DH:(i + 1) * DH],
                                                scalar1=rs[:, i:i + 1], scalar2=None,
                                                op0=AO.mult)

                # PE transposes into one PSUM tile; single ACT evacuation
                tp = pp.tile([128, EW], dt.bfloat16, tag="tp")
                for i in range(3):
                    nc.tensor.transpose(tp[:, i * DH:(i + 1) * DH],
                                        qkhat[:, i * DH:(i + 1) * DH], ident[:])
                nc.scalar.copy(qkT[:, :, st, :], tp[:].rearrange("p (i e) -> p i e", i=3))

            def attn_group(t, head_done_cb=None):
                # chunk i: i < 4t -> full k-chunk c=i over q cols [0:512)
                #          i >= 4t -> diagonal chunk c=4t+j over q cols [128j:512)
                nch = 4 * t + 4

                def chunk_info(i):
                    if i < 4 * t:
                        return i, 0
                    j = i - 4 * t
                    return 4 * t + j, 128 * j

                # flat (h, i) task list with one-task software pipelining so
                # the exp of each chunk hides under the previous chunk's
                # accumulation matmuls, across head boundaries too
                accs = {}
                pts = {}
                lbatch = []
                lacc_started = {}

                def emit_score(h, i):
                    if i == 0:
                        lacc = pp.tile([128, QT], dt.float32, tag="lacc", bufs=1)
                        oacc = pp.tile([128, QT], dt.float32, tag="oacc", bufs=1)
                        accs[h] = (lacc, oacc)
                    c, qoff = chunk_info(i)
                    wd = QT - qoff
                    stp = pp.tile([128, QT], dt.float32, tag="stp")
                    nc.tensor.matmul(stp[:, 0:wd], qkT[:, 2, c, :],
                                     qkT[:, h, 4 * t + qoff // 128:4 * t + 4, :],
                                     start=True, stop=True)
                    pt = wp.tile([128, QT], dt.bfloat16, tag="pt", bufs=6)
                    nc.scalar.activation(pt[:, 0:wd], stp[:, 0:wd], AF.Exp)
                    if i >= 4 * t:
                        nc.gpsimd.tensor_tensor(out=pt[:, 0:KC], in0=pt[:, 0:KC],
                                                in1=tri_sb[:], op=AO.mult)
                    pts[(h, i)] = (pt, c, qoff, wd)

                def emit_acc(h, i):
                    pt, c, qoff, wd = pts.pop((h, i))
                    lacc, oacc = accs[h]
                    last = i == nch - 1
                    nc.tensor.matmul(oacc[:, qoff:QT], v_sb[:, c, :], pt[:, 0:wd],
                                     start=(i == 0), stop=last)
                    if t == 0:
                        nc.tensor.matmul(lacc[:, qoff:QT], ones128[:], pt[:, 0:wd],
                                         start=(i == 0), stop=last)
                    else:
                        # defer lacc column-sums into groups of 4 emitted
                        # back-to-back with 32-wide col-tiling: the 4 matmuls
                        # run CONCURRENTLY in distinct array column groups, so
                        # the group costs ~1 chunk of moving columns, not 4.
                        # Strip j of the lacc bank accumulates chunks i%4==j
                        # (32 identical rows each); finish() sums strips and
                        # rebroadcasts with one matmul, dividing by 32.
                        lbatch.append((pt, qoff, wd))
                        if len(lbatch) == 4:
                            for j, (pt_, qoff_, wd_) in enumerate(lbatch):
                                nc.tensor.matmul(
                                    lacc[32 * j:32 * j + 32, qoff_:QT],
                                    ones128[:, 0:32], pt_[:, 0:wd_],
                                    start=(j == 0 and h not in lacc_started),
                                    stop=(last and j == 3),
                                    tile_position=(0, 32 * j))
                            lacc_started[h] = True
                            lbatch.clear()
                    if last:
                        emit_finish(h)

                def emit_finish(h):
                    # out = oacc / (lacc + exp(sink))
                    lacc, oacc = accs[h]
                    if t > 0:
                        # sum the 4 lacc strips (each 32 identical rows) and
                        # broadcast to all 128 partitions in one matmul; the
                        # 32x overcount divides out below
                        strips = wp.tile([128, QT], dt.float32, tag="tmp")
                        nc.vector.tensor_copy(strips[:], lacc[:])
                        nc.tensor.matmul(lacc[:], ones128[:], strips[:],
                                         start=True, stop=True)
                    scl = 1.0 if t == 0 else 1.0 / 32.0
                    tmp = wp.tile([128, QT], dt.float32, tag="tmp")
                    nc.vector.tensor_scalar(out=tmp[:], in0=lacc[:],
                                            scalar1=scl,
                                            scalar2=es_sb[:, h:h + 1],
                                            op0=AO.mult, op1=AO.add)
                    rr = wp.tile([128, QT], dt.float32, tag="rr")
                    nc.vector.reciprocal_approx_fast(rr[:], tmp[:])
                    att = wp.tile([128, QT], dt.bfloat16, tag="att")
                    nc.vector.tensor_tensor(out=att[:], in0=oacc[:], in1=rr[:], op=AO.mult)
                    # scatter the 4 q-subtiles to their destination-rank slots
                    r0 = 4 * (t % 2)
                    if t < 2:
                        dst = a2a_in0[:].rearrange("(r h pp) q -> pp r h q",
                                                   r=N_CORES, h=HL)[:, r0:r0 + 4, h, :]
                    else:
                        dst = b_ins[h][:].rearrange("(r pp) q -> pp r q",
                                                    r=N_CORES)[:, r0:r0 + 4, :]
                    nc.scalar.dma_start(dst, att[:].rearrange("p (j q) -> p j q", j=4))

                tasks = [(h, i) for h in range(HL) for i in range(nch)]
                emit_score(*tasks[0])
                for j in range(1, len(tasks)):
                    emit_score(*tasks[j])
                    emit_acc(*tasks[j - 1])
                    if head_done_cb is not None and tasks[j - 1] == (0, nch - 1):
                        head_done_cb()
                emit_acc(*tasks[-1])

            def emit_a2a(ins_t, outs_t):
                nc.gpsimd.collective_compute(
                    "AllToAll", AO.bypass,
                    replica_groups=[list(range(N_CORES))],
                    ins=[ins_t[:].opt()], outs=[outs_t[:].opt()],
                )

            def load_atT0():
                # on sync (nothing latency-critical sits behind it there), in
                # 4 pieces so wo's first accumulation chunk starts ~2us sooner
                src = a2a_out0[:].rearrange("(g c pp) q -> pp g c q", pp=128, g=4)
                for g in range(4):
                    nc.sync.dma_start(atT[0][:, 4 * g:4 * g + 4, :], src[:, g, :, :])

            def load_atT1(h):
                # b_outs[h] chunk r = rank r's head h = global hd chunk 2r+h
                # (2 pieces so wo's accumulation starts on the first 4 chunks
                # while the rest land)
                src = b_outs[h][:].rearrange("(g c pp) q -> pp g c q", pp=128, g=2)
                for g in range(2):
                    nc.sync.dma_start(atT[1][:, h + 8 * g:8 * g + 8:2, :],
                                      src[:, g, :, :])

            def wo_phase(p):
                # y[q128, :] = sum_c atT[p][:, c, :].T @ woT[:, c, :]
                # 4 PSUM banks cover the full 2048 douts in one accumulation
                # sweep: one stationary load per chunk feeds 2048 moving cols.
                # phase 1 accumulates even hd-chunks (from the b1 collective)
                # first so they overlap the b2 half still in flight.
                order = (list(range(NC)) if p == 0 else
                         list(range(0, NC, 2)) + list(range(1, NC, 2)))
                if p == 0:
                    # one 4-bank sweep; its evacuation overlaps phase-1 MMs
                    yps = [pp.tile([128, 512], dt.float32, tag=tg, name=f"yp0_{k}")
                           for k, tg in enumerate(("mm", "mm", "stp", "stp"))]
                    for ci, c in enumerate(order):
                        for k in range(4):
                            nc.tensor.matmul(yps[k][:], atT[p][:, c, :],
                                             wot_sb[:, c, 512 * k:512 * (k + 1)],
                                             start=(ci == 0), stop=(ci == NC - 1))
                    ysb = wp.tile([128, 2048], dt.float32, tag="ysb", bufs=1)
                    for k in range(4):
                        nc.scalar.copy(ysb[:, 512 * k:512 * (k + 1)], yps[k][:])
                    nc.scalar.dma_start(y_out[0:128, 0:1024], ysb[:, 0:1024])
                    nc.scalar.dma_start(y_out[0:128, 1024:2048], ysb[:, 1024:2048])
                else:
                    # 4 banks again, but emitted ALL-EVENS-first across both
                    # bank pairs (so the full even workload precedes the b2
                    # wait in the PE FIFO), and evacuated as two halves so the
                    # first evac+DMA runs under the second half's final MMs
                    yps = [pp.tile([128, 512], dt.float32, tag=tg, name=f"yp1_{k}")
                           for k, tg in enumerate(("mm", "mm", "stp", "stp"))]
                    half = NC // 2
                    for part in range(2):
                        sub = order[part * half:(part + 1) * half]
                        for ci, c in enumerate(sub):
                            for k in range(4):
                                nc.tensor.matmul(
                                    yps[k][:], atT[p][:, c, :],
                                    wot_sb[:, c, 512 * k:512 * (k + 1)],
                                    start=(part == 0 and ci == 0),
                                    stop=(part == 1 and ci == half - 1))
                    # final evacuation split across ACT and DVE so the two
                    # halves run in parallel right at the kernel tail
                    ysb = wp.tile([128, 2048], dt.float32, tag="ysb", bufs=1)
                    nc.scalar.copy(ysb[:, 0:512], yps[0][:])
                    nc.vector.tensor_copy(ysb[:, 1024:1536], yps[2][:])
                    nc.scalar.copy(ysb[:, 512:1024], yps[1][:])
                    nc.vector.tensor_copy(ysb[:, 1536:2048], yps[3][:])
                    nc.scalar.dma_start(y_out[128:256, 0:1024], ysb[:, 0:1024])
                    nc.scalar.dma_start(y_out[128:256, 1024:2048], ysb[:, 1024:2048])

            # ---- emission: all local attention first; wo (collective-
            # dependent) pinned last so the PE FIFO never stalls on a
            # collective while local work remains ----
            for st in range(NST):
                proj(st)
                if st in (1, 2):
                    g = st - 1
                    nc.gpsimd.dma_start(wot_sb[:, 4 * g:4 * g + 4, :],
                                        wor[:, 4 * g:4 * g + 4, :])
                if st >= 4 and st % 4 == 0:
                    attn_group(st // 4 - 1)
                    if st == 4:
                        for g in (2, 3):
                            nc.gpsimd.dma_start(wot_sb[:, 4 * g:4 * g + 4, :],
                                                wor[:, 4 * g:4 * g + 4, :])
                    if st == 8:
                        emit_a2a(a2a_in0, a2a_out0)
                    if st == 12:
                        # A2A-a certainly complete by the time the sync queue
                        # reaches this (it sits behind the whole xt stream)
                        load_atT0()
            attn_group(NQT - 1, head_done_cb=lambda: emit_a2a(b_ins[0], b_outs[0]))
            emit_a2a(b_ins[1], b_outs[1])
            with tc.tile_wait_until(1.0):
                load_atT1(0)
                wo_phase(0)
            with tc.tile_wait_until(1.1):
                load_atT1(1)
                wo_phase(1)

    nc.compile()
    return nc


def prep_inputs(x, freqs_cis, wq, wk, wv, wo, sinks):
    """Host-side sharding/layout prep. Returns in_maps for the 8 cores.

    All tensors are pre-tiled partition-major ([p, ...]) so DMAs move
    long contiguous per-partition runs.
    """
    x2 = np.ascontiguousarray(np.asarray(x, np.float32).reshape(S, D))
    xt = x2.T.astype(BF16)                                    # [D, S] = [(c p), (st s)]
    xt_h = np.ascontiguousarray(
        xt.reshape(NC, 128, NST, 128).transpose(1, 2, 0, 3).reshape(128, NST * NC * 128))

    fc = np.asarray(freqs_cis, np.float32)
    cos, sin = fc[:, :, 0], fc[:, :, 1]
    c1 = np.repeat(cos, 2, axis=1)             # [S, 128] pair-interleaved
    s1 = np.repeat(sin, 2, axis=1)
    cbar = np.concatenate([c1, c1, c1], axis=1).astype(np.float32)   # [S, 384] q0|q1|k
    sbar = np.concatenate([s1, s1, s1], axis=1).astype(np.float32)
    sbar[:, 0::2] *= -1.0                      # even outputs get -sin
    cbar_h = np.ascontiguousarray(
        cbar.reshape(NST, 128, EW).transpose(1, 0, 2).reshape(128, NST * EW)).astype(BF16)
    sbar_h = np.ascontiguousarray(
        sbar.reshape(NST, 128, EW).transpose(1, 0, 2).reshape(128, NST * EW)).astype(BF16)

    kr = np.arange(KC)[:, None]
    qr = np.arange(KC)[None, :]
    trimask = (qr >= kr).astype(np.float32).astype(BF16)      # [128, 128]

    wq = np.asarray(wq, np.float32)
    wk = np.asarray(wk, np.float32)
    wv = np.asarray(wv, np.float32)
    wo = np.asarray(wo, np.float32)
    sinks = np.asarray(sinks, np.float32)

    # full woT, identical on every core: [hd, dout] -> [p, c, dout]
    woT = np.ascontiguousarray(wo.T).astype(BF16)             # [HD=2048, D]
    wot_h = np.ascontiguousarray(
        woT.reshape(NC, 128, D).transpose(1, 0, 2).reshape(128, NC * D))

    in_maps = []
    for d in range(N_CORES):
        kv = d // 2
        es = np.exp(sinks[2 * d:2 * d + 2]).astype(np.float32)
        wqkv = np.concatenate([
            wq[d * 256:(d + 1) * 256, :].T,
            wk[kv * 128:(kv + 1) * 128, :].T,
            wv[kv * 128:(kv + 1) * 128, :].T,
        ], axis=1).astype(BF16)                               # [D, 512] = [(c p), e]
        wqkv_h = np.ascontiguousarray(
            wqkv.reshape(NC, 128, 512).transpose(1, 0, 2).reshape(128, NC * 512))
        in_maps.append({
            "xt": xt_h,
            "wqkv": wqkv_h,
            "wot": wot_h,
            "cbar": cbar_h,
            "sbar": sbar_h,
            "trimask": trimask,
            "es": np.repeat(es[None, :], 128, axis=0).astype(np.float32),
        })
    return in_maps


def assemble_output(results):
    """Interleave per-core q-row blocks: core d, phase p -> rows 1024p+128d."""
    y = np.zeros((S, D), dtype=np.float32)
    for d in range(N_CORES):
        yd = results[d]["y"]
        for p in range(2):
            y[1024 * p + 128 * d:1024 * p + 128 * d + 128, :] = yd[128 * p:128 * p + 128, :]
    return y.reshape(1, S, D)


_CACHED = {}


def kernel(x, freqs_cis, wq, wk, wv, wo, sinks):
    if "nc" not in _CACHED:
        _CACHED["nc"] = build()
    nc = _CACHED["nc"]
    in_maps = prep_inputs(x, freqs_cis, wq, wk, wv, wo, sinks)
    res = run_bass_kernel_spmd(nc, in_maps, list(range(N_CORES)), trace=False)
    return assemble_output(res.results)
